# revision 3
# baseline (speedup 1.0000x reference)
"""Multi-head attention (B=2, S=2048, D=1024, H=16, causal) on 8 TRN2 NeuronCores.

Sharding: core c -> (batch b = c//4, head-group hg = c%4). Each core:
  - projects its batch's query/key/value against a 256-row slice of Wq/Wk/Wv
    (4 heads of 64 dims),
  - runs causal attention for those 4 heads (scores computed transposed,
    exp on ACT with fused 1/8 scale, row-sums via a ones-column in V),
  - multiplies by the matching 256-column slice of Wo -> partial [2048, 1024].
Host sums the 4 partials per batch (the tensor-parallel all-reduce) and stacks.

Layout: everything is fed to the device pre-transposed ([din, tok] / [din, dout])
because the TensorE contracts over the partition dim and fp32 DMA-transpose is
not supported on TRN2.
"""

import sys

for _p in ("/opt/trn_rl_repo", "/root/.axon_site/_ro/trn_rl_repo"):
    if _p not in sys.path:
        sys.path.append(_p)

import numpy as np

import concourse.bacc as bacc
import concourse.tile as tile
import concourse.mybir as mybir
from concourse.bass import MemorySpace
from concourse.bass_utils import run_bass_kernel_spmd

f32 = mybir.dt.float32
Exp = mybir.ActivationFunctionType.Exp

B, S, D, H = 2, 2048, 1024, 16
HD = 64            # head dim
NH = 4             # heads per core
DO = NH * HD       # 256 projection out-dims per core
NCORES = 8
KI = D // 128      # 8 contraction chunks for the projections
QT = 512           # query tile (max fp32 moving free dim)
NQT = S // QT      # 4
KT = 128           # key chunk (contraction tile for PV)
NKT = S // KT      # 16

_cache: dict = {}


def _build(repeat: int = 1):
    nc = bacc.Bacc("TRN2", target_bir_lowering=False, debug=False,
                   num_devices=NCORES)

    xqT_d = nc.dram_tensor("xqT", [D, S], f32, kind="ExternalInput").ap()
    xkT_d = nc.dram_tensor("xkT", [D, S], f32, kind="ExternalInput").ap()
    xvT_d = nc.dram_tensor("xvT", [D, S], f32, kind="ExternalInput").ap()
    wqT_d = nc.dram_tensor("wqT", [D, DO], f32, kind="ExternalInput").ap()
    wkT_d = nc.dram_tensor("wkT", [D, DO], f32, kind="ExternalInput").ap()
    wvT_d = nc.dram_tensor("wvT", [D, DO], f32, kind="ExternalInput").ap()
    woT_d = nc.dram_tensor("woT", [DO, D], f32, kind="ExternalInput").ap()
    cmask_d = nc.dram_tensor("cmask", [4, 128, QT], f32, kind="ExternalInput").ap()
    out_d = nc.dram_tensor("out", [S, D], f32, kind="ExternalOutput").ap()

    with tile.TileContext(nc) as tc:
        with (
            tc.tile_pool(name="wpool", bufs=1) as wpool,
            tc.tile_pool(name="cpool", bufs=1) as cpool,
            tc.tile_pool(name="persist", bufs=1) as persist,
            tc.tile_pool(name="xin", bufs=3) as xin,
            tc.tile_pool(name="ptp", bufs=4) as ptp,
            tc.tile_pool(name="small", bufs=4) as small,
            tc.tile_pool(name="obuf", bufs=3) as obuf,
            tc.tile_pool(name="psA", bufs=3, space=MemorySpace.PSUM) as psA,
            tc.tile_pool(name="psO", bufs=2, space=MemorySpace.PSUM) as psO,
            tc.tile_pool(name="psB", bufs=2, space=MemorySpace.PSUM) as psB,
        ):
            if repeat > 1:
                with tc.For_i(0, repeat) as _i:
                    _emit(nc, wpool, cpool, persist, xin, ptp, small, obuf,
                          psA, psO, psB, xqT_d, xkT_d, xvT_d, wqT_d, wkT_d,
                          wvT_d, woT_d, cmask_d, out_d)
            else:
                _emit(nc, wpool, cpool, persist, xin, ptp, small, obuf,
                      psA, psO, psB, xqT_d, xkT_d, xvT_d, wqT_d, wkT_d,
                      wvT_d, woT_d, cmask_d, out_d)

    nc.compile()
    return nc


def _emit(nc, wpool, cpool, persist, xin, ptp, small, obuf, psA, psO, psB,
          xqT_d, xkT_d, xvT_d, wqT_d, wkT_d, wvT_d, woT_d, cmask_d, out_d):
    if True:
        if True:
            # ---- constants / weights ----
            wq_sb = wpool.tile([128, KI, DO], f32, tag="wq")
            nc.sync.dma_start(wq_sb[:], wqT_d.rearrange("(k p) n -> p k n", p=128))
            wk_sb = wpool.tile([128, KI, DO], f32, tag="wk")
            nc.sync.dma_start(wk_sb[:], wkT_d.rearrange("(k p) n -> p k n", p=128))
            wv_sb = wpool.tile([128, KI, DO], f32, tag="wv")
            nc.sync.dma_start(wv_sb[:], wvT_d.rearrange("(k p) n -> p k n", p=128))
            wo_sb = wpool.tile([128, DO // 128, D], f32, tag="wo")
            nc.sync.dma_start(wo_sb[:], woT_d.rearrange("(k p) n -> p k n", p=128))
            masks_sb = cpool.tile([128, 4, QT], f32, tag="masks")
            nc.sync.dma_start(masks_sb[:], cmask_d.rearrange("r p n -> p r n"))
            ones_sb = cpool.tile([1, HD], f32, tag="ones")
            nc.vector.memset(ones_sb[:], 1.0)

            # ---- persistent intermediates ----
            # qT/kT/oT: [256, S] stored as [128 parts, 2 chunks, S]
            #   head j lives in chunk j//2, partitions (j%2)*64 ..+64
            qT_sb = persist.tile([128, 2, S], f32, tag="qT")
            kT_sb = persist.tile([128, 2, S], f32, tag="kT")
            oT_sb = persist.tile([128, 2, S], f32, tag="oT")
            # v natural layout + ones column: [tokk part, kt, head, 65]
            v_sb = persist.tile([128, NKT, NH, HD + 1], f32, tag="v")
            nc.vector.memset(v_sb[:, :, :, HD:HD + 1], 1.0)

            # ---- phase 1: projections ----
            for t in range(NQT):
                ts = slice(t * QT, (t + 1) * QT)
                xq = xin.tile([128, KI, QT], f32, tag="xin")
                nc.sync.dma_start(
                    xq[:], xqT_d[:, ts].rearrange("(k p) n -> p k n", p=128))
                for d in range(2):
                    ps = psA.tile([128, QT], f32, tag="ps")
                    for ki in range(KI):
                        nc.tensor.matmul(
                            ps[:], wq_sb[:, ki, d * 128:(d + 1) * 128],
                            xq[:, ki, :], start=(ki == 0), stop=(ki == KI - 1))
                    nc.vector.tensor_copy(qT_sb[:, d, ts], ps[:])

                xk = xin.tile([128, KI, QT], f32, tag="xin")
                nc.sync.dma_start(
                    xk[:], xkT_d[:, ts].rearrange("(k p) n -> p k n", p=128))
                for d in range(2):
                    ps = psA.tile([128, QT], f32, tag="ps")
                    for ki in range(KI):
                        nc.tensor.matmul(
                            ps[:], wk_sb[:, ki, d * 128:(d + 1) * 128],
                            xk[:, ki, :], start=(ki == 0), stop=(ki == KI - 1))
                    nc.vector.tensor_copy(kT_sb[:, d, ts], ps[:])

                xv = xin.tile([128, KI, QT], f32, tag="xin")
                nc.sync.dma_start(
                    xv[:], xvT_d[:, ts].rearrange("(k p) n -> p k n", p=128))
                for tt in range(QT // KT):
                    kt = t * (QT // KT) + tt
                    psv = psA.tile([128, DO], f32, tag="ps")
                    for ki in range(KI):
                        nc.tensor.matmul(
                            psv[:], xv[:, ki, tt * KT:(tt + 1) * KT],
                            wv_sb[:, ki, :], start=(ki == 0), stop=(ki == KI - 1))
                    nc.vector.tensor_copy(
                        v_sb[:, kt, :, 0:HD],
                        psv[:].rearrange("p (h e) -> p h e", h=NH))

            # ---- phase 2: causal attention per head ----
            for j in range(NH):
                poff = (j % 2) * HD
                d = j // 2
                qh = qT_sb[poff:poff + HD, d, :]
                kh = kT_sb[poff:poff + HD, d, :]
                for qt in range(NQT):
                    qs = slice(qt * QT, (qt + 1) * QT)
                    nkt = (qt + 1) * (QT // KT)
                    pso = psO.tile([HD + 1, QT], f32, tag="pso")
                    for kt in range(nkt):
                        pss = psA.tile([128, QT], f32, tag="ps")
                        nc.tensor.matmul(
                            pss[:], kh[:, kt * KT:(kt + 1) * KT], qh[:, qs],
                            start=True, stop=True)
                        pt = ptp.tile([128, QT], f32, tag="pt")
                        nc.scalar.activation(pt[:], pss[:], Exp, scale=0.125)
                        r = kt - qt * (QT // KT)
                        if r >= 0:
                            nc.vector.tensor_mul(pt[:], pt[:], masks_sb[:, r, :])
                        nc.tensor.matmul(
                            pso[:], v_sb[:, kt, j, :], pt[:],
                            start=(kt == 0), stop=(kt == nkt - 1))
                    # normalize: columns of pso[0:HD] scaled by 1/rowsum
                    recip = small.tile([1, QT], f32, tag="recip")
                    nc.vector.reciprocal(recip[:], pso[HD:HD + 1, :])
                    psb = psB.tile([HD, QT], f32, tag="psb")
                    nc.tensor.matmul(psb[:], ones_sb[:], recip[:],
                                     start=True, stop=True)
                    bc = small.tile([HD, QT], f32, tag="bc")
                    nc.vector.tensor_copy(bc[:], psb[:])
                    nc.vector.tensor_mul(
                        oT_sb[poff:poff + HD, d, qs], pso[0:HD, :], bc[:])

            # ---- phase 3: output projection (partial over this core's dims) ----
            for mt in range(S // 128):
                for n in range(D // QT):
                    ps = psA.tile([128, QT], f32, tag="ps")
                    for kc in range(DO // 128):
                        nc.tensor.matmul(
                            ps[:], oT_sb[:, kc, mt * 128:(mt + 1) * 128],
                            wo_sb[:, kc, n * QT:(n + 1) * QT],
                            start=(kc == 0), stop=(kc == DO // 128 - 1))
                    ob = obuf.tile([128, QT], f32, tag="ob")
                    nc.vector.tensor_copy(ob[:], ps[:])
                    nc.sync.dma_start(
                        out_d[mt * 128:(mt + 1) * 128, n * QT:(n + 1) * QT], ob[:])


def _mask_tiles() -> np.ndarray:
    i = np.arange(128)[:, None]
    j = np.arange(QT)[None, :]
    return np.stack(
        [(j >= i + 128 * r).astype(np.float32) for r in range(4)])


def kernel(query, key, value, freqs_complex_form, mask, Wq, Wk, Wv, Wo):
    if "nc" not in _cache:
        _cache["nc"] = _build()
    nc = _cache["nc"]

    query = np.asarray(query, np.float32)
    key = np.asarray(key, np.float32)
    value = np.asarray(value, np.float32)
    Wq = np.asarray(Wq, np.float32)
    Wk = np.asarray(Wk, np.float32)
    Wv = np.asarray(Wv, np.float32)
    Wo = np.asarray(Wo, np.float32)
    cm = _mask_tiles()

    in_maps = []
    for c in range(NCORES):
        b, hg = divmod(c, NCORES // B)
        sl = slice(hg * DO, (hg + 1) * DO)
        in_maps.append({
            "xqT": np.ascontiguousarray(query[b].T),
            "xkT": np.ascontiguousarray(key[b].T),
            "xvT": np.ascontiguousarray(value[b].T),
            "wqT": np.ascontiguousarray(Wq[sl].T),
            "wkT": np.ascontiguousarray(Wk[sl].T),
            "wvT": np.ascontiguousarray(Wv[sl].T),
            "woT": np.ascontiguousarray(Wo[:, sl].T),
            "cmask": cm,
        })

    res = run_bass_kernel_spmd(nc, in_maps, list(range(NCORES)))
    parts = [res.results[c]["out"] for c in range(NCORES)]
    npg = NCORES // B
    return np.stack(
        [np.sum(parts[b * npg:(b + 1) * npg], axis=0) for b in range(B)]
    ).astype(np.float32)


# revision 6
# speedup vs baseline: 2.3577x; 2.3577x over previous
"""Multi-head attention (B=2, S=2048, D=1024, H=16, causal) on 8 TRN2 NeuronCores.

Sharding: core c -> (batch b = c//4, head-group hg = c%4). Each core:
  - projects its batch's query/key/value against a 256-row slice of Wq/Wk/Wv
    (4 heads of 64 dims),
  - runs causal attention for those 4 heads (scores computed transposed,
    exp on ACT with fused 1/8 scale, row-sums via a ones-column in V),
  - multiplies by the matching 256-column slice of Wo -> partial [2048, 1024].
Host sums the 4 partials per batch (the tensor-parallel all-reduce) and stacks.

Layout: operands are fed to the device pre-transposed ([din, tok] / [din, dout])
because the TensorE contracts over the partition dim and fp32 DMA-transpose is
not supported on TRN2.

Precision: matmuls run in float32r (TRN2's full-rate fp32 mode: inputs rounded
to 11 mantissa bits, fp32 accumulate in PSUM). Plain fp32 matmuls run at 1/4
rate. Inputs are pre-rounded on the host (round-half-up at bit 12, matching
hardware), which the BIR verifier accepts for direct DMA->matmul use.
"""

import sys

for _p in ("/opt/trn_rl_repo", "/root/.axon_site/_ro/trn_rl_repo"):
    if _p not in sys.path:
        sys.path.append(_p)

import numpy as np

import concourse.bacc as bacc
import concourse.tile as tile
import concourse.mybir as mybir
from concourse.bass import MemorySpace
from concourse.bass_utils import run_bass_kernel_spmd

f32 = mybir.dt.float32
f32r = mybir.dt.float32r
Exp = mybir.ActivationFunctionType.Exp

B, S, D, H = 2, 2048, 1024, 16
HD = 64            # head dim
NH = 4             # heads per core
DO = NH * HD       # 256 projection out-dims per core
NCORES = 8
KI = D // 128      # 8 contraction chunks for the projections
QT = 512           # query tile
NQT = S // QT      # 4
KT = 128           # key chunk (contraction tile for PV)
NKT = S // KT      # 16

_cache: dict = {}


def _build(repeat: int = 1):
    nc = bacc.Bacc("TRN2", target_bir_lowering=False, debug=False,
                   num_devices=NCORES)

    xqT_d = nc.dram_tensor("xqT", [D, S], f32r, kind="ExternalInput").ap()
    xkT_d = nc.dram_tensor("xkT", [D, S], f32r, kind="ExternalInput").ap()
    xvT_d = nc.dram_tensor("xvT", [D, S], f32r, kind="ExternalInput").ap()
    wqT_d = nc.dram_tensor("wqT", [D, DO], f32r, kind="ExternalInput").ap()
    wkT_d = nc.dram_tensor("wkT", [D, DO], f32r, kind="ExternalInput").ap()
    wvT_d = nc.dram_tensor("wvT", [D, DO], f32r, kind="ExternalInput").ap()
    woT_d = nc.dram_tensor("woT", [DO, D], f32r, kind="ExternalInput").ap()
    cmask_d = nc.dram_tensor("cmask", [4, 128, QT], f32r, kind="ExternalInput").ap()
    out_d = nc.dram_tensor("out", [S, D], f32, kind="ExternalOutput").ap()

    with tile.TileContext(nc) as tc:
        with (
            tc.tile_pool(name="wpool", bufs=1) as wpool,
            tc.tile_pool(name="cpool", bufs=1) as cpool,
            tc.tile_pool(name="persist", bufs=1) as persist,
            tc.tile_pool(name="xin", bufs=3) as xin,
            tc.tile_pool(name="ptp", bufs=4) as ptp,
            tc.tile_pool(name="small", bufs=4) as small,
            tc.tile_pool(name="obuf", bufs=3) as obuf,
            tc.tile_pool(name="psA", bufs=3, space=MemorySpace.PSUM) as psA,
            tc.tile_pool(name="psO", bufs=2, space=MemorySpace.PSUM) as psO,
            tc.tile_pool(name="psB", bufs=2, space=MemorySpace.PSUM) as psB,
        ):
            pools = (nc, wpool, cpool, persist, xin, ptp, small, obuf,
                     psA, psO, psB, xqT_d, xkT_d, xvT_d, wqT_d, wkT_d,
                     wvT_d, woT_d, cmask_d, out_d)
            if repeat > 1:
                with tc.For_i(0, repeat):
                    _emit(*pools)
            else:
                _emit(*pools)

    nc.compile()
    return nc


def _emit(nc, wpool, cpool, persist, xin, ptp, small, obuf, psA, psO, psB,
          xqT_d, xkT_d, xvT_d, wqT_d, wkT_d, wvT_d, woT_d, cmask_d, out_d):
    # ---- constants / weights ----
    wq_sb = wpool.tile([128, KI, DO], f32r, tag="wq")
    nc.sync.dma_start(wq_sb[:], wqT_d.rearrange("(k p) n -> p k n", p=128))
    wk_sb = wpool.tile([128, KI, DO], f32r, tag="wk")
    nc.sync.dma_start(wk_sb[:], wkT_d.rearrange("(k p) n -> p k n", p=128))
    wv_sb = wpool.tile([128, KI, DO], f32r, tag="wv")
    nc.sync.dma_start(wv_sb[:], wvT_d.rearrange("(k p) n -> p k n", p=128))
    wo_sb = wpool.tile([128, DO // 128, D], f32r, tag="wo")
    nc.sync.dma_start(wo_sb[:], woT_d.rearrange("(k p) n -> p k n", p=128))
    masks_sb = cpool.tile([128, 4, QT], f32r, tag="masks")
    nc.sync.dma_start(masks_sb[:], cmask_d.rearrange("r p n -> p r n"))
    ones_f = cpool.tile([1, HD], f32, tag="ones_f")
    nc.vector.memset(ones_f[:], 1.0)
    ones_sb = cpool.tile([1, HD], f32r, tag="ones")
    nc.vector.tensor_copy(ones_sb[:], ones_f[:])

    # ---- persistent intermediates ----
    # qT/kT/oT: [256, S] stored as [128 parts, 2 chunks, S]
    #   head j lives in chunk j//2, partitions (j%2)*64 ..+64
    qT_sb = persist.tile([128, 2, S], f32r, tag="qT")
    kT_sb = persist.tile([128, 2, S], f32r, tag="kT")
    oT_sb = persist.tile([128, 2, S], f32r, tag="oT")
    # v natural layout + ones column: [tokk part, kt, head, 65]
    v_sb = persist.tile([128, NKT, NH, HD + 1], f32r, tag="v")
    vones_f = cpool.tile([128, NKT * NH], f32, tag="vones_f")
    nc.vector.memset(vones_f[:], 1.0)
    nc.vector.tensor_copy(
        v_sb[:, :, :, HD],
        vones_f[:].rearrange("p (a b) -> p a b", a=NKT))

    # ---- phase 1: projections ----
    for t in range(NQT):
        ts = slice(t * QT, (t + 1) * QT)
        xq = xin.tile([128, KI, QT], f32r, tag="xin")
        nc.sync.dma_start(
            xq[:], xqT_d[:, ts].rearrange("(k p) n -> p k n", p=128))
        for d in range(2):
            ps = psA.tile([128, QT], f32, tag="ps")
            for ki in range(KI):
                nc.tensor.matmul(
                    ps[:], wq_sb[:, ki, d * 128:(d + 1) * 128],
                    xq[:, ki, :], start=(ki == 0), stop=(ki == KI - 1))
            nc.vector.tensor_copy(qT_sb[:, d, ts], ps[:])

        xk = xin.tile([128, KI, QT], f32r, tag="xin")
        nc.sync.dma_start(
            xk[:], xkT_d[:, ts].rearrange("(k p) n -> p k n", p=128))
        for d in range(2):
            ps = psA.tile([128, QT], f32, tag="ps")
            for ki in range(KI):
                nc.tensor.matmul(
                    ps[:], wk_sb[:, ki, d * 128:(d + 1) * 128],
                    xk[:, ki, :], start=(ki == 0), stop=(ki == KI - 1))
            nc.vector.tensor_copy(kT_sb[:, d, ts], ps[:])

        xv = xin.tile([128, KI, QT], f32r, tag="xin")
        nc.sync.dma_start(
            xv[:], xvT_d[:, ts].rearrange("(k p) n -> p k n", p=128))
        for tt in range(QT // KT):
            kt = t * (QT // KT) + tt
            psv = psA.tile([128, DO], f32, tag="ps")
            for ki in range(KI):
                nc.tensor.matmul(
                    psv[:], xv[:, ki, tt * KT:(tt + 1) * KT],
                    wv_sb[:, ki, :], start=(ki == 0), stop=(ki == KI - 1))
            nc.vector.tensor_copy(
                v_sb[:, kt, :, 0:HD],
                psv[:].rearrange("p (h e) -> p h e", h=NH))

    # ---- phase 2: causal attention per head ----
    for j in range(NH):
        poff = (j % 2) * HD
        d = j // 2
        qh = qT_sb[poff:poff + HD, d, :]
        kh = kT_sb[poff:poff + HD, d, :]
        for qt in range(NQT):
            qs = slice(qt * QT, (qt + 1) * QT)
            nkt = (qt + 1) * (QT // KT)
            pso = psO.tile([HD + 1, QT], f32, tag="pso")
            for kt in range(nkt):
                pss = psA.tile([128, QT], f32, tag="ps")
                nc.tensor.matmul(
                    pss[:], kh[:, kt * KT:(kt + 1) * KT], qh[:, qs],
                    start=True, stop=True)
                pt = ptp.tile([128, QT], f32r, tag="pt")
                nc.scalar.activation(pt[:], pss[:], Exp, scale=0.125)
                r = kt - qt * (QT // KT)
                if r >= 0:
                    nc.vector.tensor_mul(pt[:], pt[:], masks_sb[:, r, :])
                nc.tensor.matmul(
                    pso[:], v_sb[:, kt, j, :], pt[:],
                    start=(kt == 0), stop=(kt == nkt - 1))
            # normalize: columns of pso[0:HD] scaled by 1/rowsum
            recip = small.tile([1, QT], f32, tag="recip")
            nc.vector.reciprocal(recip[:], pso[HD:HD + 1, :])
            recir = small.tile([1, QT], f32r, tag="recir")
            nc.vector.tensor_copy(recir[:], recip[:])
            psb = psB.tile([HD, QT], f32, tag="psb")
            nc.tensor.matmul(psb[:], ones_sb[:], recir[:],
                             start=True, stop=True)
            bc = small.tile([HD, QT], f32, tag="bc")
            nc.vector.tensor_copy(bc[:], psb[:])
            nc.vector.tensor_mul(
                oT_sb[poff:poff + HD, d, qs], pso[0:HD, :], bc[:])

    # ---- phase 3: output projection (partial over this core's dims) ----
    for mt in range(S // 128):
        for n in range(D // QT):
            ps = psA.tile([128, QT], f32, tag="ps")
            for kc in range(DO // 128):
                nc.tensor.matmul(
                    ps[:], oT_sb[:, kc, mt * 128:(mt + 1) * 128],
                    wo_sb[:, kc, n * QT:(n + 1) * QT],
                    start=(kc == 0), stop=(kc == DO // 128 - 1))
            ob = obuf.tile([128, QT], f32, tag="ob")
            nc.vector.tensor_copy(ob[:], ps[:])
            nc.sync.dma_start(
                out_d[mt * 128:(mt + 1) * 128, n * QT:(n + 1) * QT], ob[:])


def _round_f32r(x: np.ndarray) -> np.ndarray:
    """Round fp32 to float32r (round-half-up at mantissa bit 12, matching HW)."""
    b = np.ascontiguousarray(x, np.float32).view(np.uint32).astype(np.uint64)
    b = (b + (1 << 11)) & np.uint64(0xFFFFF000)
    return b.astype(np.uint32).view(np.float32)


def _mask_tiles() -> np.ndarray:
    i = np.arange(128)[:, None]
    j = np.arange(QT)[None, :]
    return np.stack(
        [(j >= i + 128 * r).astype(np.float32) for r in range(4)])


def make_in_maps(query, key, value, Wq, Wk, Wv, Wo):
    query = np.asarray(query, np.float32)
    key = np.asarray(key, np.float32)
    value = np.asarray(value, np.float32)
    Wq = np.asarray(Wq, np.float32)
    Wk = np.asarray(Wk, np.float32)
    Wv = np.asarray(Wv, np.float32)
    Wo = np.asarray(Wo, np.float32)
    cm = _mask_tiles()
    in_maps = []
    for c in range(NCORES):
        b, hg = divmod(c, NCORES // B)
        sl = slice(hg * DO, (hg + 1) * DO)
        in_maps.append({
            "xqT": _round_f32r(query[b].T),
            "xkT": _round_f32r(key[b].T),
            "xvT": _round_f32r(value[b].T),
            "wqT": _round_f32r(Wq[sl].T),
            "wkT": _round_f32r(Wk[sl].T),
            "wvT": _round_f32r(Wv[sl].T),
            "woT": _round_f32r(Wo[:, sl].T),
            "cmask": cm,
        })
    return in_maps


def kernel(query, key, value, freqs_complex_form, mask, Wq, Wk, Wv, Wo):
    if "nc" not in _cache:
        _cache["nc"] = _build()
    nc = _cache["nc"]
    in_maps = make_in_maps(query, key, value, Wq, Wk, Wv, Wo)
    res = run_bass_kernel_spmd(nc, in_maps, list(range(NCORES)))
    parts = [res.results[c]["out"] for c in range(NCORES)]
    npg = NCORES // B
    return np.stack(
        [np.sum(parts[b * npg:(b + 1) * npg], axis=0) for b in range(B)]
    ).astype(np.float32)


# revision 9
# speedup vs baseline: 2.3604x; 1.0011x over previous
"""Multi-head attention (B=2, S=2048, D=1024, H=16, causal) on 8 TRN2 NeuronCores.

Sharding: core c -> (batch b = c//4, head-group hg = c%4). Each core:
  - projects its batch's query/key/value against a 256-row slice of Wq/Wk/Wv
    (4 heads of 64 dims),
  - runs causal attention for those 4 heads (scores computed transposed,
    exp on ACT with fused 1/8 scale, row-sums via a ones-column in V),
  - multiplies by the matching 256-column slice of Wo -> partial [2048, 1024].
Host sums the 4 partials per batch (the tensor-parallel all-reduce) and stacks.

Layout: operands are fed to the device pre-transposed ([din, tok] / [din, dout])
because the TensorE contracts over the partition dim and fp32 DMA-transpose is
not supported on TRN2.

Precision: matmuls run in float32r (TRN2's full-rate fp32 mode: inputs rounded
to 11 mantissa bits, fp32 accumulate in PSUM). Plain fp32 matmuls run at 1/4
rate. Inputs are pre-rounded on the host (round-half-up at bit 12, matching
hardware), which the BIR verifier accepts for direct DMA->matmul use.
"""

import sys

for _p in ("/opt/trn_rl_repo", "/root/.axon_site/_ro/trn_rl_repo"):
    if _p not in sys.path:
        sys.path.append(_p)

import numpy as np

import concourse.bacc as bacc
import concourse.tile as tile
import concourse.mybir as mybir
from concourse.bass import MemorySpace
from concourse.bass_utils import run_bass_kernel_spmd

f32 = mybir.dt.float32
f32r = mybir.dt.float32r
Exp = mybir.ActivationFunctionType.Exp

B, S, D, H = 2, 2048, 1024, 16
HD = 64            # head dim
NH = 4             # heads per core
DO = NH * HD       # 256 projection out-dims per core
NCORES = 8
KI = D // 128      # 8 contraction chunks for the projections
QT = 512           # query tile
NQT = S // QT      # 4
KT = 128           # key chunk (contraction tile for PV)
NKT = S // KT      # 16

_cache: dict = {}


def _build(repeat: int = 1):
    nc = bacc.Bacc("TRN2", target_bir_lowering=False, debug=False,
                   num_devices=NCORES)

    xqT_d = nc.dram_tensor("xqT", [D, S], f32r, kind="ExternalInput").ap()
    xkT_d = nc.dram_tensor("xkT", [D, S], f32r, kind="ExternalInput").ap()
    xvT_d = nc.dram_tensor("xvT", [D, S], f32r, kind="ExternalInput").ap()
    wqT_d = nc.dram_tensor("wqT", [D, DO], f32r, kind="ExternalInput").ap()
    wkT_d = nc.dram_tensor("wkT", [D, DO], f32r, kind="ExternalInput").ap()
    wvT_d = nc.dram_tensor("wvT", [D, DO], f32r, kind="ExternalInput").ap()
    woT_d = nc.dram_tensor("woT", [DO, D], f32r, kind="ExternalInput").ap()
    cmask_d = nc.dram_tensor("cmask", [128, KT], f32r, kind="ExternalInput").ap()
    out_d = nc.dram_tensor("out", [S, D], f32, kind="ExternalOutput").ap()

    with tile.TileContext(nc) as tc:
        with (
            tc.tile_pool(name="wpool", bufs=1) as wpool,
            tc.tile_pool(name="cpool", bufs=1) as cpool,
            tc.tile_pool(name="persist", bufs=1) as persist,
            tc.tile_pool(name="xin", bufs=4) as xin,
            tc.tile_pool(name="ptp", bufs=4) as ptp,
            tc.tile_pool(name="small", bufs=4) as small,
            tc.tile_pool(name="obuf", bufs=3) as obuf,
            tc.tile_pool(name="psA", bufs=3, space=MemorySpace.PSUM) as psA,
            tc.tile_pool(name="psO", bufs=2, space=MemorySpace.PSUM) as psO,
            tc.tile_pool(name="psB", bufs=2, space=MemorySpace.PSUM) as psB,
        ):
            pools = (nc, wpool, cpool, persist, xin, ptp, small, obuf,
                     psA, psO, psB, xqT_d, xkT_d, xvT_d, wqT_d, wkT_d,
                     wvT_d, woT_d, cmask_d, out_d)
            if repeat > 1:
                with tc.For_i(0, repeat):
                    _emit(*pools)
            else:
                _emit(*pools)

    nc.compile()
    return nc


def _emit(nc, wpool, cpool, persist, xin, ptp, small, obuf, psA, psO, psB,
          xqT_d, xkT_d, xvT_d, wqT_d, wkT_d, wvT_d, woT_d, cmask_d, out_d):
    NT = QT // KT  # 4 key chunks per token block

    # ---- constants / weights ----
    wq_sb = wpool.tile([128, KI, DO], f32r, tag="wq")
    nc.sync.dma_start(wq_sb[:], wqT_d.rearrange("(k p) n -> p k n", p=128))
    wk_sb = wpool.tile([128, KI, DO], f32r, tag="wk")
    nc.sync.dma_start(wk_sb[:], wkT_d.rearrange("(k p) n -> p k n", p=128))
    wv_sb = wpool.tile([128, KI, DO], f32r, tag="wv")
    nc.sync.dma_start(wv_sb[:], wvT_d.rearrange("(k p) n -> p k n", p=128))
    wo_sb = wpool.tile([128, DO // 128, D], f32r, tag="wo")
    nc.sync.dma_start(wo_sb[:], woT_d.rearrange("(k p) n -> p k n", p=128))
    # single triangular mask tile (j >= i), applied to the first 128 cols
    # of the column-restricted diagonal tiles
    tri_sb = cpool.tile([128, KT], f32r, tag="tri")
    nc.sync.dma_start(tri_sb[:], cmask_d)
    ones_f = cpool.tile([1, HD], f32, tag="ones_f")
    nc.vector.memset(ones_f[:], 1.0)
    ones_sb = cpool.tile([1, HD], f32r, tag="ones")
    nc.vector.tensor_copy(ones_sb[:], ones_f[:])
    vones_f = cpool.tile([128, NT * NH], f32, tag="vones_f")
    nc.vector.memset(vones_f[:], 1.0)

    # ---- per-block persistent intermediates ----
    # qT/kT/oT blocks: [256, QT] as [128 parts, 2 chunks, QT]
    #   head j lives in chunk j//2, partitions (j%2)*64 ..+64
    qTt = [persist.tile([128, 2, QT], f32r, tag=f"qT{t}", name=f"qT{t}")
           for t in range(NQT)]
    kTt = [persist.tile([128, 2, QT], f32r, tag=f"kT{t}", name=f"kT{t}")
           for t in range(NQT)]
    oTt = [persist.tile([128, 2, QT], f32r, tag=f"oT{t}", name=f"oT{t}")
           for t in range(NQT)]
    # v blocks, natural layout + ones column: [tokk part, ktc, head, 65]
    vt = [persist.tile([128, NT, NH, HD + 1], f32r, tag=f"v{t}", name=f"v{t}")
          for t in range(NQT)]

    for t in range(NQT):
        ts = slice(t * QT, (t + 1) * QT)

        # ---- projections for token block t ----
        xq = xin.tile([128, KI, QT], f32r, tag="xin")
        nc.sync.dma_start(
            xq[:], xqT_d[:, ts].rearrange("(k p) n -> p k n", p=128))
        for d in range(2):
            ps = psA.tile([128, QT], f32, tag="ps")
            for ki in range(KI):
                nc.tensor.matmul(
                    ps[:], wq_sb[:, ki, d * 128:(d + 1) * 128],
                    xq[:, ki, :], start=(ki == 0), stop=(ki == KI - 1))
            nc.vector.tensor_copy(qTt[t][:, d, :], ps[:])

        xk = xin.tile([128, KI, QT], f32r, tag="xin")
        nc.sync.dma_start(
            xk[:], xkT_d[:, ts].rearrange("(k p) n -> p k n", p=128))
        for d in range(2):
            ps = psA.tile([128, QT], f32, tag="ps")
            for ki in range(KI):
                nc.tensor.matmul(
                    ps[:], wk_sb[:, ki, d * 128:(d + 1) * 128],
                    xk[:, ki, :], start=(ki == 0), stop=(ki == KI - 1))
            nc.vector.tensor_copy(kTt[t][:, d, :], ps[:])

        xv = xin.tile([128, KI, QT], f32r, tag="xin")
        nc.sync.dma_start(
            xv[:], xvT_d[:, ts].rearrange("(k p) n -> p k n", p=128))
        nc.vector.tensor_copy(
            vt[t][:, :, :, HD], vones_f[:].rearrange("p (a b) -> p a b", a=NT))
        for tt in range(NT):
            psv = psA.tile([128, DO], f32, tag="ps")
            for ki in range(KI):
                nc.tensor.matmul(
                    psv[:], xv[:, ki, tt * KT:(tt + 1) * KT],
                    wv_sb[:, ki, :], start=(ki == 0), stop=(ki == KI - 1))
            nc.vector.tensor_copy(
                vt[t][:, tt, :, 0:HD],
                psv[:].rearrange("p (h e) -> p h e", h=NH))

        # ---- causal attention for query block qt = t, all heads ----
        qt = t
        for j in range(NH):
            poff = (j % 2) * HD
            d = j // 2
            qh = qTt[qt][poff:poff + HD, d, :]
            nkt = (qt + 1) * NT
            pso = psO.tile([HD + 1, QT], f32, tag="pso")
            for kt in range(nkt):
                r = kt - qt * NT
                co = max(r, 0) * KT          # column offset into the q block
                w = QT - co                  # restricted width
                kh = kTt[kt // NT][poff:poff + HD, d,
                                   (kt % NT) * KT:(kt % NT + 1) * KT]
                pss = psA.tile([128, QT], f32, tag="ps")
                nc.tensor.matmul(
                    pss[:, 0:w], kh, qh[:, co:QT], start=True, stop=True)
                pt = ptp.tile([128, QT], f32r, tag="pt")
                nc.scalar.activation(pt[:, 0:w], pss[:, 0:w], Exp, scale=0.125)
                if r >= 0:
                    nc.vector.tensor_mul(
                        pt[:, 0:KT], pt[:, 0:KT], tri_sb[:])
                nc.tensor.matmul(
                    pso[:, co:QT], vt[kt // NT][:, kt % NT, j, :], pt[:, 0:w],
                    start=(kt == 0), stop=(kt == nkt - 1))
            # normalize: columns of pso[0:HD] scaled by 1/rowsum
            recip = small.tile([1, QT], f32, tag="recip")
            nc.vector.reciprocal(recip[:], pso[HD:HD + 1, :])
            recir = small.tile([1, QT], f32r, tag="recir")
            nc.vector.tensor_copy(recir[:], recip[:])
            psb = psB.tile([HD, QT], f32, tag="psb")
            nc.tensor.matmul(psb[:], ones_sb[:], recir[:],
                             start=True, stop=True)
            bc = small.tile([HD, QT], f32, tag="bc")
            nc.vector.tensor_copy(bc[:], psb[:])
            nc.vector.tensor_mul(
                oTt[qt][poff:poff + HD, d, :], pso[0:HD, :], bc[:])

        # ---- output projection for token block t (partial dims) ----
        for mtt in range(NT):
            mt = t * NT + mtt
            for n in range(D // QT):
                ps = psA.tile([128, QT], f32, tag="ps")
                for kc in range(DO // 128):
                    nc.tensor.matmul(
                        ps[:], oTt[t][:, kc, mtt * KT:(mtt + 1) * KT],
                        wo_sb[:, kc, n * QT:(n + 1) * QT],
                        start=(kc == 0), stop=(kc == DO // 128 - 1))
                ob = obuf.tile([128, QT], f32, tag="ob")
                nc.scalar.copy(ob[:], ps[:])
                nc.sync.dma_start(
                    out_d[mt * 128:(mt + 1) * 128, n * QT:(n + 1) * QT], ob[:])


def _round_f32r(x: np.ndarray) -> np.ndarray:
    """Round fp32 to float32r (round-half-up at mantissa bit 12, matching HW)."""
    b = np.ascontiguousarray(x, np.float32).view(np.uint32).astype(np.uint64)
    b = (b + (1 << 11)) & np.uint64(0xFFFFF000)
    return b.astype(np.uint32).view(np.float32)


def _mask_tiles() -> np.ndarray:
    i = np.arange(128)[:, None]
    j = np.arange(KT)[None, :]
    return (j >= i).astype(np.float32)


def make_in_maps(query, key, value, Wq, Wk, Wv, Wo):
    query = np.asarray(query, np.float32)
    key = np.asarray(key, np.float32)
    value = np.asarray(value, np.float32)
    Wq = np.asarray(Wq, np.float32)
    Wk = np.asarray(Wk, np.float32)
    Wv = np.asarray(Wv, np.float32)
    Wo = np.asarray(Wo, np.float32)
    cm = _mask_tiles()
    in_maps = []
    for c in range(NCORES):
        b, hg = divmod(c, NCORES // B)
        sl = slice(hg * DO, (hg + 1) * DO)
        in_maps.append({
            "xqT": _round_f32r(query[b].T),
            "xkT": _round_f32r(key[b].T),
            "xvT": _round_f32r(value[b].T),
            "wqT": _round_f32r(Wq[sl].T),
            "wkT": _round_f32r(Wk[sl].T),
            "wvT": _round_f32r(Wv[sl].T),
            "woT": _round_f32r(Wo[:, sl].T),
            "cmask": cm,
        })
    return in_maps


def kernel(query, key, value, freqs_complex_form, mask, Wq, Wk, Wv, Wo):
    if "nc" not in _cache:
        _cache["nc"] = _build()
    nc = _cache["nc"]
    in_maps = make_in_maps(query, key, value, Wq, Wk, Wv, Wo)
    res = run_bass_kernel_spmd(nc, in_maps, list(range(NCORES)))
    parts = [res.results[c]["out"] for c in range(NCORES)]
    npg = NCORES // B
    return np.stack(
        [np.sum(parts[b * npg:(b + 1) * npg], axis=0) for b in range(B)]
    ).astype(np.float32)


# revision 16
# speedup vs baseline: 2.4328x; 1.0307x over previous
"""Multi-head attention (B=2, S=2048, D=1024, H=16, causal) on 8 TRN2 NeuronCores.

Sharding: core c -> (batch b = c//4, head-group hg = c%4). Each core:
  - projects its batch's query/key/value against a 256-row slice of Wq/Wk/Wv
    (4 heads of 64 dims),
  - runs causal attention for those 4 heads (scores computed transposed,
    exp on ACT with fused 1/8 scale, row-sums via a ones-column in V),
  - multiplies by the matching 256-column slice of Wo -> partial [2048, 1024].
Host sums the 4 partials per batch (the tensor-parallel all-reduce) and stacks.

Layout: operands are fed to the device pre-transposed ([din, tok] / [din, dout])
because the TensorE contracts over the partition dim and fp32 DMA-transpose is
not supported on TRN2.

Precision: matmuls run in float32r (TRN2's full-rate fp32 mode: inputs rounded
to 11 mantissa bits, fp32 accumulate in PSUM). Plain fp32 matmuls run at 1/4
rate. Inputs are pre-rounded on the host (round-half-up at bit 12, matching
hardware), which the BIR verifier accepts for direct DMA->matmul use.
"""

import sys

for _p in ("/opt/trn_rl_repo", "/root/.axon_site/_ro/trn_rl_repo"):
    if _p not in sys.path:
        sys.path.append(_p)

import numpy as np

import concourse.bacc as bacc
import concourse.tile as tile
import concourse.mybir as mybir
from concourse.bass import MemorySpace
from concourse.bass_utils import run_bass_kernel_spmd

f32 = mybir.dt.float32
f32r = mybir.dt.float32r
Exp = mybir.ActivationFunctionType.Exp

B, S, D, H = 2, 2048, 1024, 16
HD = 64            # head dim
NH = 4             # heads per core
DO = NH * HD       # 256 projection out-dims per core
NCORES = 8
KI = D // 128      # 8 contraction chunks for the projections
QT = 512           # query tile
NQT = S // QT      # 4
KT = 128           # key chunk (contraction tile for PV)
NKT = S // KT      # 16

_cache: dict = {}

# ablation switches for perf experiments (leave defaults for production)
_opts = {"attn": True, "outproj": True, "exp": True, "oproj_copy": "dve",
         "mask": True, "norm": True, "proj_copy": "dve"}


def _build(repeat: int = 1):
    nc = bacc.Bacc("TRN2", target_bir_lowering=False, debug=False,
                   num_devices=NCORES)

    xqT_d = nc.dram_tensor("xqT", [D, S], f32r, kind="ExternalInput").ap()
    xkT_d = nc.dram_tensor("xkT", [D, S], f32r, kind="ExternalInput").ap()
    xvT_d = nc.dram_tensor("xvT", [D, S], f32r, kind="ExternalInput").ap()
    wqT_d = nc.dram_tensor("wqT", [D, DO], f32r, kind="ExternalInput").ap()
    wkT_d = nc.dram_tensor("wkT", [D, DO], f32r, kind="ExternalInput").ap()
    wvT_d = nc.dram_tensor("wvT", [D, DO], f32r, kind="ExternalInput").ap()
    woT_d = nc.dram_tensor("woT", [DO, D], f32r, kind="ExternalInput").ap()
    cmask_d = nc.dram_tensor("cmask", [128, KT], f32r, kind="ExternalInput").ap()
    out_d = nc.dram_tensor("out", [S, D], f32, kind="ExternalOutput").ap()

    with tile.TileContext(nc) as tc:
        with (
            tc.tile_pool(name="wpool", bufs=1) as wpool,
            tc.tile_pool(name="cpool", bufs=1) as cpool,
            tc.tile_pool(name="persist", bufs=1) as persist,
            tc.tile_pool(name="xin", bufs=4) as xin,
            tc.tile_pool(name="ptp", bufs=4) as ptp,
            tc.tile_pool(name="small", bufs=4) as small,
            tc.tile_pool(name="obuf", bufs=3) as obuf,
            tc.tile_pool(name="psA", bufs=3, space=MemorySpace.PSUM) as psA,
            tc.tile_pool(name="psO", bufs=4, space=MemorySpace.PSUM) as psO,
        ):
            pools = (nc, wpool, cpool, persist, xin, ptp, small, obuf,
                     psA, psO, xqT_d, xkT_d, xvT_d, wqT_d, wkT_d,
                     wvT_d, woT_d, cmask_d, out_d)
            if repeat > 1:
                with tc.For_i(0, repeat):
                    _emit(*pools)
            else:
                _emit(*pools)

    nc.compile()
    return nc


def _emit(nc, wpool, cpool, persist, xin, ptp, small, obuf, psA, psO,
          xqT_d, xkT_d, xvT_d, wqT_d, wkT_d, wvT_d, woT_d, cmask_d, out_d):
    NT = QT // KT  # 4 key chunks per token block

    # ---- constants / weights ----
    wq_sb = wpool.tile([128, KI, DO], f32r, tag="wq")
    nc.sync.dma_start(wq_sb[:], wqT_d.rearrange("(k p) n -> p k n", p=128))
    wk_sb = wpool.tile([128, KI, DO], f32r, tag="wk")
    nc.sync.dma_start(wk_sb[:], wkT_d.rearrange("(k p) n -> p k n", p=128))
    wv_sb = wpool.tile([128, KI, DO], f32r, tag="wv")
    nc.sync.dma_start(wv_sb[:], wvT_d.rearrange("(k p) n -> p k n", p=128))
    wo_sb = wpool.tile([128, DO // 128, D], f32r, tag="wo")
    nc.sync.dma_start(wo_sb[:], woT_d.rearrange("(k p) n -> p k n", p=128))
    # single triangular mask tile (j >= i), applied to the first 128 cols
    # of the column-restricted diagonal tiles
    tri_sb = cpool.tile([128, KT], f32r, tag="tri")
    nc.sync.dma_start(tri_sb[:], cmask_d)
    ones_f = cpool.tile([1, HD], f32, tag="ones_f")
    nc.vector.memset(ones_f[:], 1.0)
    ones_sb = cpool.tile([1, HD], f32r, tag="ones")
    nc.vector.tensor_copy(ones_sb[:], ones_f[:])
    vones_f = cpool.tile([128, NT * NH], f32, tag="vones_f")
    nc.vector.memset(vones_f[:], 1.0)

    # ---- per-block persistent intermediates ----
    # qT/kT/oT blocks: [256, QT] as [128 parts, 2 chunks, QT]
    #   head j lives in chunk j//2, partitions (j%2)*64 ..+64
    qTt = [persist.tile([128, 2, QT], f32r, tag=f"qT{t}", name=f"qT{t}")
           for t in range(NQT)]
    kTt = [persist.tile([128, 2, QT], f32r, tag=f"kT{t}", name=f"kT{t}")
           for t in range(NQT)]
    oTt = [persist.tile([128, 2, QT], f32r, tag=f"oT{t}", name=f"oT{t}")
           for t in range(NQT)]
    # v blocks, natural layout + ones column: [tokk part, ktc, head, 65]
    vt = [persist.tile([128, NT, NH, HD + 1], f32r, tag=f"v{t}", name=f"v{t}")
          for t in range(NQT)]

    for t in range(NQT):
        ts = slice(t * QT, (t + 1) * QT)

        # ---- projections for token block t ----
        xq = xin.tile([128, KI, QT], f32r, tag="xin")
        nc.sync.dma_start(
            xq[:], xqT_d[:, ts].rearrange("(k p) n -> p k n", p=128))
        for d in range(2):
            ps = psA.tile([128, QT], f32, tag="ps")
            for ki in range(KI):
                nc.tensor.matmul(
                    ps[:], wq_sb[:, ki, d * 128:(d + 1) * 128],
                    xq[:, ki, :], start=(ki == 0), stop=(ki == KI - 1))
            nc.vector.tensor_copy(qTt[t][:, d, :], ps[:])

        xk = xin.tile([128, KI, QT], f32r, tag="xin")
        nc.sync.dma_start(
            xk[:], xkT_d[:, ts].rearrange("(k p) n -> p k n", p=128))
        for d in range(2):
            ps = psA.tile([128, QT], f32, tag="ps")
            for ki in range(KI):
                nc.tensor.matmul(
                    ps[:], wk_sb[:, ki, d * 128:(d + 1) * 128],
                    xk[:, ki, :], start=(ki == 0), stop=(ki == KI - 1))
            nc.vector.tensor_copy(kTt[t][:, d, :], ps[:])

        xv = xin.tile([128, KI, QT], f32r, tag="xin")
        nc.sync.dma_start(
            xv[:], xvT_d[:, ts].rearrange("(k p) n -> p k n", p=128))
        nc.vector.tensor_copy(
            vt[t][:, :, :, HD], vones_f[:].rearrange("p (a b) -> p a b", a=NT))
        for tt in range(NT):
            psv = psA.tile([128, DO], f32, tag="ps")
            for ki in range(KI):
                nc.tensor.matmul(
                    psv[:], xv[:, ki, tt * KT:(tt + 1) * KT],
                    wv_sb[:, ki, :], start=(ki == 0), stop=(ki == KI - 1))
            nc.vector.tensor_copy(
                vt[t][:, tt, :, 0:HD],
                psv[:].rearrange("p (h e) -> p h e", h=NH))

        # ---- causal attention for query block qt = t, all heads ----
        qt = t
        for j in range(NH if _opts["attn"] else 0):
            poff = (j % 2) * HD
            d = j // 2
            qh = qTt[qt][poff:poff + HD, d, :]
            nkt = (qt + 1) * NT
            pso = psO.tile([HD + 1, QT], f32, tag="pso")
            pending = None  # software pipeline: PV lags scores/exp by one
            for kt in range(nkt):
                r = kt - qt * NT
                co = max(r, 0) * KT          # column offset into the q block
                w = QT - co                  # restricted width
                kh = kTt[kt // NT][poff:poff + HD, d,
                                   (kt % NT) * KT:(kt % NT + 1) * KT]
                pss = psA.tile([128, QT], f32, tag="ps")
                nc.tensor.matmul(
                    pss[:, 0:w], kh, qh[:, co:QT], start=True, stop=True)
                pt = ptp.tile([128, QT], f32r, tag="pt")
                if _opts["exp"]:
                    nc.scalar.activation(pt[:, 0:w], pss[:, 0:w], Exp,
                                         scale=0.125)
                else:
                    nc.scalar.copy(pt[:, 0:w], pss[:, 0:w])
                if r >= 0 and _opts["mask"]:
                    nc.vector.tensor_mul(
                        pt[:, 0:KT], pt[:, 0:KT], tri_sb[:])
                if pending is not None:
                    pkt, pco, pw, ppt = pending
                    nc.tensor.matmul(
                        pso[:, pco:QT],
                        vt[pkt // NT][:, pkt % NT, j, :], ppt[:, 0:pw],
                        start=(pkt == 0), stop=False)
                pending = (kt, co, w, pt)
            pkt, pco, pw, ppt = pending
            nc.tensor.matmul(
                pso[:, pco:QT], vt[pkt // NT][:, pkt % NT, j, :], ppt[:, 0:pw],
                start=(pkt == 0), stop=True)
            # normalize: columns of pso[0:HD] scaled by 1/rowsum
            if _opts["norm"]:
                recip = small.tile([1, QT], f32, tag="recip")
                nc.vector.reciprocal(recip[:], pso[HD:HD + 1, :])
                recir = small.tile([1, QT], f32r, tag="recir")
                nc.vector.tensor_copy(recir[:], recip[:])
                psb = psA.tile([HD, QT], f32, tag="ps")
                nc.tensor.matmul(psb[:], ones_sb[:], recir[:],
                                 start=True, stop=True)
                bc = small.tile([HD, QT], f32, tag="bc")
                nc.vector.tensor_copy(bc[:], psb[:])
                nc.vector.tensor_mul(
                    oTt[qt][poff:poff + HD, d, :], pso[0:HD, :], bc[:])
            else:
                nc.vector.tensor_copy(
                    oTt[qt][poff:poff + HD, d, :], pso[0:HD, :])

        # ---- output projection for token block t (partial dims) ----
        for mtt in range(NT if _opts["outproj"] else 0):
            mt = t * NT + mtt
            for n in range(D // QT):
                ps = psA.tile([128, QT], f32, tag="ps")
                for kc in range(DO // 128):
                    nc.tensor.matmul(
                        ps[:], oTt[t][:, kc, mtt * KT:(mtt + 1) * KT],
                        wo_sb[:, kc, n * QT:(n + 1) * QT],
                        start=(kc == 0), stop=(kc == DO // 128 - 1))
                ob = obuf.tile([128, QT], f32, tag="ob")
                if _opts["oproj_copy"] == "act":
                    nc.scalar.copy(ob[:], ps[:])
                else:
                    nc.vector.tensor_copy(ob[:], ps[:])
                nc.sync.dma_start(
                    out_d[mt * 128:(mt + 1) * 128, n * QT:(n + 1) * QT], ob[:])


def _round_f32r(x: np.ndarray) -> np.ndarray:
    """Round fp32 to float32r (round-half-up at mantissa bit 12, matching HW)."""
    b = np.ascontiguousarray(x, np.float32).view(np.uint32).astype(np.uint64)
    b = (b + (1 << 11)) & np.uint64(0xFFFFF000)
    return b.astype(np.uint32).view(np.float32)


def _mask_tiles() -> np.ndarray:
    i = np.arange(128)[:, None]
    j = np.arange(KT)[None, :]
    return (j >= i).astype(np.float32)


def make_in_maps(query, key, value, Wq, Wk, Wv, Wo):
    query = np.asarray(query, np.float32)
    key = np.asarray(key, np.float32)
    value = np.asarray(value, np.float32)
    Wq = np.asarray(Wq, np.float32)
    Wk = np.asarray(Wk, np.float32)
    Wv = np.asarray(Wv, np.float32)
    Wo = np.asarray(Wo, np.float32)
    cm = _mask_tiles()
    in_maps = []
    for c in range(NCORES):
        b, hg = divmod(c, NCORES // B)
        sl = slice(hg * DO, (hg + 1) * DO)
        in_maps.append({
            "xqT": _round_f32r(query[b].T),
            "xkT": _round_f32r(key[b].T),
            "xvT": _round_f32r(value[b].T),
            "wqT": _round_f32r(Wq[sl].T),
            "wkT": _round_f32r(Wk[sl].T),
            "wvT": _round_f32r(Wv[sl].T),
            "woT": _round_f32r(Wo[:, sl].T),
            "cmask": cm,
        })
    return in_maps


def kernel(query, key, value, freqs_complex_form, mask, Wq, Wk, Wv, Wo):
    if "nc" not in _cache:
        _cache["nc"] = _build()
    nc = _cache["nc"]
    in_maps = make_in_maps(query, key, value, Wq, Wk, Wv, Wo)
    res = run_bass_kernel_spmd(nc, in_maps, list(range(NCORES)))
    parts = [res.results[c]["out"] for c in range(NCORES)]
    npg = NCORES // B
    return np.stack(
        [np.sum(parts[b * npg:(b + 1) * npg], axis=0) for b in range(B)]
    ).astype(np.float32)


# revision 17
# speedup vs baseline: 2.5171x; 1.0347x over previous
"""Multi-head attention (B=2, S=2048, D=1024, H=16, causal) on 8 TRN2 NeuronCores.

Sharding: core c -> (batch b = c//4, head-group hg = c%4). Each core:
  - projects its batch's query/key/value against a 256-row slice of Wq/Wk/Wv
    (4 heads of 64 dims),
  - runs causal attention for those 4 heads (scores computed transposed,
    exp on ACT with fused 1/8 scale, row-sums via a ones-column in V),
  - multiplies by the matching 256-column slice of Wo -> partial [2048, 1024].
Host sums the 4 partials per batch (the tensor-parallel all-reduce) and stacks.

Layout: operands are fed to the device pre-transposed ([din, tok] / [din, dout])
because the TensorE contracts over the partition dim and fp32 DMA-transpose is
not supported on TRN2.

Precision: matmuls run in float32r (TRN2's full-rate fp32 mode: inputs rounded
to 11 mantissa bits, fp32 accumulate in PSUM). Plain fp32 matmuls run at 1/4
rate. Inputs are pre-rounded on the host (round-half-up at bit 12, matching
hardware), which the BIR verifier accepts for direct DMA->matmul use.
"""

import sys

for _p in ("/opt/trn_rl_repo", "/root/.axon_site/_ro/trn_rl_repo"):
    if _p not in sys.path:
        sys.path.append(_p)

import numpy as np

import concourse.bacc as bacc
import concourse.tile as tile
import concourse.mybir as mybir
from concourse.bass import MemorySpace
from concourse.bass_utils import run_bass_kernel_spmd

f32 = mybir.dt.float32
f32r = mybir.dt.float32r
Exp = mybir.ActivationFunctionType.Exp

B, S, D, H = 2, 2048, 1024, 16
HD = 64            # head dim
NH = 4             # heads per core
DO = NH * HD       # 256 projection out-dims per core
NCORES = 8
KI = D // 128      # 8 contraction chunks for the projections
QT = 512           # query tile
NQT = S // QT      # 4
KT = 128           # key chunk (contraction tile for PV)
NKT = S // KT      # 16

_cache: dict = {}

# ablation switches for perf experiments (leave defaults for production)
_opts = {"attn": True, "outproj": True, "exp": True, "oproj_copy": "dve",
         "mask": True, "norm": True, "proj_copy": "dve"}


def _build(repeat: int = 1):
    nc = bacc.Bacc("TRN2", target_bir_lowering=False, debug=False,
                   num_devices=NCORES)

    xqT_d = nc.dram_tensor("xqT", [D, S], f32r, kind="ExternalInput").ap()
    xkT_d = nc.dram_tensor("xkT", [D, S], f32r, kind="ExternalInput").ap()
    xvT_d = nc.dram_tensor("xvT", [D, S], f32r, kind="ExternalInput").ap()
    wqT_d = nc.dram_tensor("wqT", [D, DO], f32r, kind="ExternalInput").ap()
    wkT_d = nc.dram_tensor("wkT", [D, DO], f32r, kind="ExternalInput").ap()
    wvT_d = nc.dram_tensor("wvT", [D, DO], f32r, kind="ExternalInput").ap()
    woT_d = nc.dram_tensor("woT", [DO, D], f32r, kind="ExternalInput").ap()
    cmask_d = nc.dram_tensor("cmask", [128, KT], f32r, kind="ExternalInput").ap()
    out_d = nc.dram_tensor("out", [S, D], f32, kind="ExternalOutput").ap()

    with tile.TileContext(nc) as tc:
        with (
            tc.tile_pool(name="wpool", bufs=1) as wpool,
            tc.tile_pool(name="cpool", bufs=1) as cpool,
            tc.tile_pool(name="persist", bufs=1) as persist,
            tc.tile_pool(name="xin", bufs=4) as xin,
            tc.tile_pool(name="ptp", bufs=4) as ptp,
            tc.tile_pool(name="small", bufs=4) as small,
            tc.tile_pool(name="obuf", bufs=3) as obuf,
            tc.tile_pool(name="psA", bufs=3, space=MemorySpace.PSUM) as psA,
            tc.tile_pool(name="psO", bufs=4, space=MemorySpace.PSUM) as psO,
        ):
            pools = (nc, wpool, cpool, persist, xin, ptp, small, obuf,
                     psA, psO, xqT_d, xkT_d, xvT_d, wqT_d, wkT_d,
                     wvT_d, woT_d, cmask_d, out_d)
            if repeat > 1:
                with tc.For_i(0, repeat):
                    _emit(*pools)
            else:
                _emit(*pools)

    nc.compile()
    return nc


def _emit(nc, wpool, cpool, persist, xin, ptp, small, obuf, psA, psO,
          xqT_d, xkT_d, xvT_d, wqT_d, wkT_d, wvT_d, woT_d, cmask_d, out_d):
    NT = QT // KT  # 4 key chunks per token block

    # ---- constants / weights ----
    wq_sb = wpool.tile([128, KI, DO], f32r, tag="wq")
    nc.sync.dma_start(wq_sb[:], wqT_d.rearrange("(k p) n -> p k n", p=128))
    wk_sb = wpool.tile([128, KI, DO], f32r, tag="wk")
    nc.sync.dma_start(wk_sb[:], wkT_d.rearrange("(k p) n -> p k n", p=128))
    wv_sb = wpool.tile([128, KI, DO], f32r, tag="wv")
    nc.sync.dma_start(wv_sb[:], wvT_d.rearrange("(k p) n -> p k n", p=128))
    wo_sb = wpool.tile([128, DO // 128, D], f32r, tag="wo")
    nc.sync.dma_start(wo_sb[:], woT_d.rearrange("(k p) n -> p k n", p=128))
    # single triangular mask tile (j >= i), applied to the first 128 cols
    # of the column-restricted diagonal tiles
    tri_sb = cpool.tile([128, KT], f32r, tag="tri")
    nc.sync.dma_start(tri_sb[:], cmask_d)
    ones_f = cpool.tile([1, HD], f32, tag="ones_f")
    nc.vector.memset(ones_f[:], 1.0)
    ones_sb = cpool.tile([1, HD], f32r, tag="ones")
    nc.vector.tensor_copy(ones_sb[:], ones_f[:])
    vones_f = cpool.tile([128, NT * NH], f32, tag="vones_f")
    nc.vector.memset(vones_f[:], 1.0)

    # ---- per-block persistent intermediates ----
    # qT/kT/oT blocks: [256, QT] as [128 parts, 2 chunks, QT]
    #   head j lives in chunk j//2, partitions (j%2)*64 ..+64
    qTt = [persist.tile([128, 2, QT], f32r, tag=f"qT{t}", name=f"qT{t}")
           for t in range(NQT)]
    kTt = [persist.tile([128, 2, QT], f32r, tag=f"kT{t}", name=f"kT{t}")
           for t in range(NQT)]
    oTt = [persist.tile([128, 2, QT], f32r, tag=f"oT{t}", name=f"oT{t}")
           for t in range(NQT)]
    # v blocks, natural layout + ones column: [tokk part, ktc, head, 65]
    vt = [persist.tile([128, NT, NH, HD + 1], f32r, tag=f"v{t}", name=f"v{t}")
          for t in range(NQT)]

    for t in range(NQT):
        ts = slice(t * QT, (t + 1) * QT)

        # ---- projections for token block t ----
        xq = xin.tile([128, KI, QT], f32r, tag="xin")
        nc.sync.dma_start(
            xq[:], xqT_d[:, ts].rearrange("(k p) n -> p k n", p=128))
        for d in range(2):
            ps = psA.tile([128, QT], f32, tag="ps")
            for ki in range(KI):
                nc.tensor.matmul(
                    ps[:], wq_sb[:, ki, d * 128:(d + 1) * 128],
                    xq[:, ki, :], start=(ki == 0), stop=(ki == KI - 1))
            nc.vector.tensor_copy(qTt[t][:, d, :], ps[:])

        xk = xin.tile([128, KI, QT], f32r, tag="xin")
        nc.sync.dma_start(
            xk[:], xkT_d[:, ts].rearrange("(k p) n -> p k n", p=128))
        for d in range(2):
            ps = psA.tile([128, QT], f32, tag="ps")
            for ki in range(KI):
                nc.tensor.matmul(
                    ps[:], wk_sb[:, ki, d * 128:(d + 1) * 128],
                    xk[:, ki, :], start=(ki == 0), stop=(ki == KI - 1))
            nc.vector.tensor_copy(kTt[t][:, d, :], ps[:])

        xv = xin.tile([128, KI, QT], f32r, tag="xin")
        nc.sync.dma_start(
            xv[:], xvT_d[:, ts].rearrange("(k p) n -> p k n", p=128))
        nc.vector.tensor_copy(
            vt[t][:, :, :, HD], vones_f[:].rearrange("p (a b) -> p a b", a=NT))
        for tt in range(NT):
            psv = psA.tile([128, DO], f32, tag="ps")
            for ki in range(KI):
                nc.tensor.matmul(
                    psv[:], xv[:, ki, tt * KT:(tt + 1) * KT],
                    wv_sb[:, ki, :], start=(ki == 0), stop=(ki == KI - 1))
            nc.vector.tensor_copy(
                vt[t][:, tt, :, 0:HD],
                psv[:].rearrange("p (h e) -> p h e", h=NH))

        # ---- causal attention for query block qt = t, all heads ----
        # all 4 heads' accumulation loops first (psO holds 4 accumulators),
        # then the normalize chains together so no chain blocks the engine
        # streams mid-block.
        qt = t
        psos = []
        for j in range(NH if _opts["attn"] else 0):
            poff = (j % 2) * HD
            d = j // 2
            qh = qTt[qt][poff:poff + HD, d, :]
            nkt = (qt + 1) * NT
            pso = psO.tile([HD + 1, QT], f32, tag="pso")
            pending = None  # software pipeline: PV lags scores/exp by one
            for kt in range(nkt):
                r = kt - qt * NT
                co = max(r, 0) * KT          # column offset into the q block
                w = QT - co                  # restricted width
                kh = kTt[kt // NT][poff:poff + HD, d,
                                   (kt % NT) * KT:(kt % NT + 1) * KT]
                pss = psA.tile([128, QT], f32, tag="ps")
                nc.tensor.matmul(
                    pss[:, 0:w], kh, qh[:, co:QT], start=True, stop=True)
                pt = ptp.tile([128, QT], f32r, tag="pt")
                if _opts["exp"]:
                    nc.scalar.activation(pt[:, 0:w], pss[:, 0:w], Exp,
                                         scale=0.125)
                else:
                    nc.scalar.copy(pt[:, 0:w], pss[:, 0:w])
                if r >= 0 and _opts["mask"]:
                    nc.vector.tensor_mul(
                        pt[:, 0:KT], pt[:, 0:KT], tri_sb[:])
                if pending is not None:
                    pkt, pco, pw, ppt = pending
                    nc.tensor.matmul(
                        pso[:, pco:QT],
                        vt[pkt // NT][:, pkt % NT, j, :], ppt[:, 0:pw],
                        start=(pkt == 0), stop=False)
                pending = (kt, co, w, pt)
            pkt, pco, pw, ppt = pending
            nc.tensor.matmul(
                pso[:, pco:QT], vt[pkt // NT][:, pkt % NT, j, :], ppt[:, 0:pw],
                start=(pkt == 0), stop=True)
            psos.append((j, pso))
        # normalize: columns of pso[0:HD] scaled by 1/rowsum
        for j, pso in psos:
            poff = (j % 2) * HD
            d = j // 2
            if _opts["norm"]:
                recir = small.tile([1, QT], f32r, tag="recir")
                with nc.allow_low_precision(reason="f32r normalization scale"):
                    nc.vector.reciprocal(recir[:], pso[HD:HD + 1, :])
                psb = psA.tile([HD, QT], f32, tag="ps")
                nc.tensor.matmul(psb[:], ones_sb[:], recir[:],
                                 start=True, stop=True)
                bc = small.tile([HD, QT], f32, tag="bc")
                nc.vector.tensor_copy(bc[:], psb[:])
                nc.vector.tensor_mul(
                    oTt[qt][poff:poff + HD, d, :], pso[0:HD, :], bc[:])
            else:
                nc.vector.tensor_copy(
                    oTt[qt][poff:poff + HD, d, :], pso[0:HD, :])

        # ---- output projection for token block t (partial dims) ----
        for mtt in range(NT if _opts["outproj"] else 0):
            mt = t * NT + mtt
            for n in range(D // QT):
                ps = psA.tile([128, QT], f32, tag="ps")
                for kc in range(DO // 128):
                    nc.tensor.matmul(
                        ps[:], oTt[t][:, kc, mtt * KT:(mtt + 1) * KT],
                        wo_sb[:, kc, n * QT:(n + 1) * QT],
                        start=(kc == 0), stop=(kc == DO // 128 - 1))
                ob = obuf.tile([128, QT], f32, tag="ob")
                if _opts["oproj_copy"] == "act":
                    nc.scalar.copy(ob[:], ps[:])
                else:
                    nc.vector.tensor_copy(ob[:], ps[:])
                nc.sync.dma_start(
                    out_d[mt * 128:(mt + 1) * 128, n * QT:(n + 1) * QT], ob[:])


def _round_f32r(x: np.ndarray) -> np.ndarray:
    """Round fp32 to float32r (round-half-up at mantissa bit 12, matching HW)."""
    b = np.ascontiguousarray(x, np.float32).view(np.uint32).astype(np.uint64)
    b = (b + (1 << 11)) & np.uint64(0xFFFFF000)
    return b.astype(np.uint32).view(np.float32)


def _mask_tiles() -> np.ndarray:
    i = np.arange(128)[:, None]
    j = np.arange(KT)[None, :]
    return (j >= i).astype(np.float32)


def make_in_maps(query, key, value, Wq, Wk, Wv, Wo):
    query = np.asarray(query, np.float32)
    key = np.asarray(key, np.float32)
    value = np.asarray(value, np.float32)
    Wq = np.asarray(Wq, np.float32)
    Wk = np.asarray(Wk, np.float32)
    Wv = np.asarray(Wv, np.float32)
    Wo = np.asarray(Wo, np.float32)
    cm = _mask_tiles()
    in_maps = []
    for c in range(NCORES):
        b, hg = divmod(c, NCORES // B)
        sl = slice(hg * DO, (hg + 1) * DO)
        in_maps.append({
            "xqT": _round_f32r(query[b].T),
            "xkT": _round_f32r(key[b].T),
            "xvT": _round_f32r(value[b].T),
            "wqT": _round_f32r(Wq[sl].T),
            "wkT": _round_f32r(Wk[sl].T),
            "wvT": _round_f32r(Wv[sl].T),
            "woT": _round_f32r(Wo[:, sl].T),
            "cmask": cm,
        })
    return in_maps


def kernel(query, key, value, freqs_complex_form, mask, Wq, Wk, Wv, Wo):
    if "nc" not in _cache:
        _cache["nc"] = _build()
    nc = _cache["nc"]
    in_maps = make_in_maps(query, key, value, Wq, Wk, Wv, Wo)
    res = run_bass_kernel_spmd(nc, in_maps, list(range(NCORES)))
    parts = [res.results[c]["out"] for c in range(NCORES)]
    npg = NCORES // B
    return np.stack(
        [np.sum(parts[b * npg:(b + 1) * npg], axis=0) for b in range(B)]
    ).astype(np.float32)
